# revision 45
# baseline (speedup 1.0000x reference)
"""Sparse-attention kernel for Trainium2, 8-core SPMD (queries sharded).

Computes out = softmax(Q @ K^T / sqrt(D) + m) @ V for
Q,K,V: [8192, 64] f32, m: [8192, 8192] f32.

Strategy (per core c over query shard q_c = rows [c*1024, (c+1)*1024)):
  Everything is computed in transposed (S^T) layout so that the exp output
  lands directly in the [key, query] orientation the PV matmul needs.

  Key idea vs the additive-mask formulation: softmax(s + m) uses
  exp(s + m) = exp(s) * exp(m), and the softmax ratio is shift-invariant.
  The host ships em = exp(m)^T in f16 (same bytes as m in f16), the device
  exps the *pure* QK scores straight out of PSUM (no mask add in PSUM at
  all), and the mask is applied as an all-SBUF f16 multiply on the DVE
  (2x mode), which is far off the critical path. This removes the PE
  identity-matmul and the DVE f32 PSUM add, and leaves PSUM holding only
  QK scores so ScalarE activations can batch chunks.

  Chunks are processed in [pair, single] triplets: chunks 3k/3k+1 land
  in the PSUM score tile st_b [128,2,1024] (4 banks), chunk 3k+2 in
  st_a [128,1024] (2 banks), + two O^T h-half accumulators (1 bank
  each) = 8 banks exactly. Each pair is ONE batched [2,1024] ACTIVATE, so
  ScalarE runs 1966+1114 = 3080ns per 3 chunks (vs 3x1114); the measured
  period is ~3104ns/triplet, pinned by the invariant chain
  [pair-ACT end -> two slot-recycling QKs -> next pair-ACT]. Two separate score
  tiles matter: the dependency tracker treats PSUM reads as RMW, totally
  ordering all accesses to one tile by emission order; the split lets the
  single-chain (QK->ACT_s) overlap the pair-chain (QK,QK->ACT_p).
  Emission is explicitly software-pipelined (QKs one triplet ahead of
  their group, each ACT immediately followed by the QK that recycles its
  slot) because the in-order PE queue otherwise head-blocks on PVs.

  Host-side sharding prep (layout/dtype only, plus exp(m)):
    em   = exp(m[q_c, :]).T      [8192, 1024] f16
    qt   = Q[q_c].T / 8          [64, 1024]   f16   (pad rows memset on-chip)
    kt   = K.T                   [64, 8192]   f16   (pad rows memset on-chip)
    va   = [V | 1]               [128, CK*66] f16   (col 64 of each chunk = 1
                                                     -> row 64 of O^T = sum(P))
  Device, per k-chunk j (128 keys):
    S^T[j]  = kt_j.T @ qt                    (PE -> PSUM score tile)
    E^T     = exp(S^T - 1)                   (ScalarE, PSUM -> SBUF f16)
    P^T[j]  = E^T[j] * em_j                  (DVE f16 multiply, 2x mode)
    O^T    += va_j.T @ P^T[j]                (PSUM [65, 2x512], accumulated)
  Ramp: qt leads the sync DMA queue, kt goes in fine-grained slices on the
  gpsimd queue (first QK only needs 32KB), em pairs + va slices stream on
  sync; zero-pad rows are memset on gpsimd instead of DMAing hosted zeros
  (the early wire is the ramp bottleneck). A short throwaway-matmul burst
  warms the PE HAM clock gate and hands off into real QK work. Tail:
  chunk 63's exp/mult are h-split, the two O^T h-halves drain through
  ScalarE/DVE copies + sync DMAs in parallel; the host divides num/sum.
"""

import numpy as np

P = 128
D = 64
NQ = 8192
NK = 8192
N_CORES = 8
VF = 66  # vaug chunk stride (65 cols used, padded for alignment)
FDIM = 512  # matmul moving free dim (one PSUM bank of f32)
NSLOT = 3  # PSUM score-ring depth (chunks in flight)
FEW = 256  # columns per single-chunk routed to the DVE fast-exp path
FE_A = 1024.0 / 0.6931471805599453  # f16 Schraudolph scale (2^10/ln2)
FE_B = 15 * 1024 - 47.0 - FE_A      # bias - sigma - the exp(-1) shift

_nc_cache = {}
_patched = [False]


def _install_tile_patch():
    """No-op placeholder kept for API stability (see _split_excess_waits)."""
    _patched[0] = True


def _split_excess_waits(nc, max_waits=1):
    """Walrus in this toolchain rejects instructions carrying more than one
    inline sync-wait command. Move excess waits onto same-engine NOPs
    inserted immediately before the instruction (the engine executes them
    in order, so the barrier semantics are preserved)."""
    import concourse.mybir as mybir

    for fn in nc.m.functions:
        for blk in fn.blocks:
            idx = 0
            while idx < len(blk.instructions):
                inst = blk.instructions[idx]
                si = inst.sync_info
                waits = list(si.on_wait) if si is not None and si.on_wait else []
                if len(waits) <= max_waits:
                    idx += 1
                    continue
                updates = list(si.on_update) if si.on_update else []
                keep = waits[-max_waits:]
                rest = waits[:-max_waits]
                inst.sync_info = mybir.SyncInfo(on_wait=keep, on_update=updates)
                n_nops = 0
                for i in range(0, len(rest), max_waits):
                    nop = mybir.InstNoOp(
                        name=nc.get_next_instruction_name(), ins=[], outs=[]
                    )
                    nop.engine = inst.engine
                    nop.sync_info = mybir.SyncInfo(
                        on_wait=rest[i:i + max_waits], on_update=[]
                    )
                    nc.register_instruction(nop)
                    blk.instructions.insert(idx + n_nops, nop)
                    n_nops += 1
                idx += n_nops + 1


def _build_nc(qsh, nk, mt_bufs=20, e_bufs=8, light_tail=True, kp=P):
    import concourse.bass as bass
    import concourse.mybir as mybir
    import concourse.tile as tile

    dt = mybir.dt
    ck = nk // P          # number of 128-key chunks
    npair = ck // 2       # em DMAs move two chunks at a time
    nh = qsh // FDIM      # number of 512-query column blocks
    nks = 16              # kt/va DMA split count (spread over first pairs)
    assert qsh % FDIM == 0 and nk % (2 * P) == 0 and nk % nks == 0 and (ck * VF) % nks == 0

    nc = bass.Bass()
    em = nc.declare_dram_parameter("em", [nk, qsh], dt.float16, isOutput=False)
    qt = nc.declare_dram_parameter("qt", [D, qsh], dt.float16, isOutput=False)
    kt = nc.declare_dram_parameter("kt", [D, nk], dt.float16, isOutput=False)
    va = nc.declare_dram_parameter("va", [P, ck * VF], dt.float16, isOutput=False)
    out = nc.declare_dram_parameter("ot_out", [D + 1, qsh], dt.float16, isOutput=True)

    em_pairs = em.rearrange("(pp c p) q -> pp p c q", c=2, p=P)  # [npair, 128, 2, qsh]

    if light_tail:
        _install_light_tail()

    # activation groups in a repeating [pair, single] triplet pattern:
    # pairs always land on ring slots (0,1) (tile st_b) and singles on
    # slot 2 (tile st_a), so every pair batches into one [2, qsh]
    # ACTIVATE: 3080ns of ScalarE work per 3 chunks vs 3x1114.
    groups = []
    for k in range(ck // 3):
        groups += [[3 * k, 3 * k + 1], [3 * k + 2]]
    groups += [[c] for c in range(3 * (ck // 3), ck)]

    with tile.TileContext(nc) as tc:
        with (
            tc.tile_pool(name="const", bufs=1) as cpool,
            tc.tile_pool(name="mtp", bufs=mt_bufs) as mtp,
            tc.tile_pool(name="ep", bufs=e_bufs) as epool,
            tc.tile_pool(name="fxp", bufs=3) as fxp,
            tc.tile_pool(name="ptp", bufs=e_bufs) as ptp,
            tc.tile_pool(name="tail", bufs=1) as tailp,
            tc.tile_pool(name="stp", bufs=1, space="PSUM") as stp,
            tc.tile_pool(name="otp", bufs=1, space="PSUM") as otp,
        ):
            # --- earliest DMAs first: nothing on the device gates these.
            # qt leads the sync queue (it gates the very first QK); the em
            # stream follows. kt/qt are host-padded to 128 rows (K=64
            # row-group matmuls keep the PE in its low-power half-array
            # mode -- measured 2x slower -- and an on-chip pad memset costs
            # ~7us of DVE time).
            qt_sb = cpool.tile([kp, qsh], dt.float16)
            nc.sync.dma_start(qt_sb[0:D, 0:FDIM], qt[:, 0:FDIM])
            nc.sync.dma_start(qt_sb[0:D, FDIM:qsh], qt[:, FDIM:qsh])

            mt_tiles = {}
            mt_tiles[0] = mtp.tile([P, 2, qsh], dt.float16, name="mt0", tag="mt")
            nc.sync.dma_start(mt_tiles[0][:], em_pairs[0])

            # kt slice plan: two 2-chunk slices up front (the first QK
            # only needs 32KB, not 0.5MB), then 4-chunk slices. Issued on
            # gpsimd ahead of the consuming chunk; va rides the sync queue
            # interleaved with the em pairs.
            kt_sb = cpool.tile([kp, nk], dt.float16)
            kt_slices = [(0, 3 * P), (3 * P, 4 * P)] + [
                (4 * P * i, 4 * P * (i + 1)) for i in range(1, ck // 4)
            ]
            kt_issue = {}  # chunk j -> kt slice indices (slices 0-2 below)
            for si in range(3, len(kt_slices)):
                kt_issue.setdefault(max(2, 4 * (si - 1) - 8), []).append(si)
            # DMA issues first, pad memsets after: the memsets are not
            # needed until the QKs (~13us) but each one delays the next
            # DMA issue on the in-order gpsimd queue
            for si in (0, 1, 2):
                a, b = kt_slices[si]
                nc.gpsimd.dma_start(kt_sb[0:D, a:b], kt[:, a:b])
            nc.gpsimd.memset(qt_sb[D:kp, :], 0.0)
            for si in (0, 1, 2):
                a, b = kt_slices[si]
                nc.gpsimd.memset(kt_sb[D:kp, a:b], 0.0)

            va_sb = cpool.tile([P, ck * VF], dt.float16)
            nva = 16
            vs = (ck * VF) // nva
            nc.sync.dma_start(va_sb[:, 0:vs], va[:, 0:vs])

            # --- warm-up: exp spline tables + PE HAM, riding the DMA ramp ---
            warm = cpool.tile([1, 2], dt.float32)
            nc.vector.memset(warm[:], 0.0)
            nc.scalar.activation(
                warm[:], warm[:], mybir.ActivationFunctionType.Exp
            )

            # per-partition bias vector holding the -1 softmax shift
            # (overflow headroom for the f16 exp products)
            nbias = cpool.tile([P, 1], dt.float32)
            nc.vector.memset(nbias[:], -1.0)

            # Score ring as TWO tiles: slot 0 (singles) and slots 1-2
            # (pairs). The dependency tracker treats PSUM reads as RMW, so
            # all accesses to one tile are totally ordered by emission;
            # separate tiles let the single-chain (QK -> ACT_s) overlap the
            # pair-chain (QK,QK -> ACT_p) instead of serializing PE against
            # ScalarE.
            st_a = stp.tile([P, qsh], dt.float32, name="st_a")      # 2 banks
            st_b = stp.tile([P, 2, qsh], dt.float32, name="st_b")   # 4 banks
            ot_h = [
                otp.tile([D + 1, FDIM], dt.float32, name="ot_h0"),
                otp.tile([D + 1, FDIM], dt.float32, name="ot_h1"),
            ]  # 1 bank each; separate tiles so the h0 drain chain doesn't
            # wait on h1's final PV (PSUM accesses are totally ordered
            # per tile)

            wz = cpool.tile([P, P], dt.float16)
            nc.vector.memset(wz[:], 0.0)
            for _ in range(5):
                nc.tensor.matmul(
                    st_a[:, 0:P], wz[:], wz[:],
                    start=True, stop=True, skip_group_check=True,
                )

            def st_ap(j, sl):
                s = j % NSLOT
                return st_a[:, sl] if s == 2 else st_b[:, s, sl]

            def emit_qk(j):
                # N=512 per matmul: a matmul dst may not cross a PSUM bank
                # boundary (hard HW constraint, verified), so each chunk's
                # scores take two FDIM-wide matmuls
                ktj = kt_sb[:, j * P:(j + 1) * P]
                for h in range(nh):
                    sl = slice(h * FDIM, (h + 1) * FDIM)
                    nc.tensor.matmul(
                        st_ap(j, sl), ktj, qt_sb[:, sl],
                        start=True, stop=True, skip_group_check=True,
                    )

            e_tiles = {}

            def fast_off(j):
                # single non-edge chunks offload FEW query-columns to a DVE
                # int16-Schraudolph exp; the offset alternates per triplet
                # so no query row concentrates too much approximation error
                # (A Schraudolph int16 fast-exp path on the DVE was
                # measured here: it saves ScalarE time on paper but gains
                # ~0 wall-clock in the fast clock state while spending most
                # of the 2e-2 error budget (1.78e-2) -- disabled.)
                return None

            def emit_act(g):
                e_t = epool.tile([P, 2, qsh], dt.float16, name=f"e{g[0]}", tag="e")
                if len(g) == 2:
                    e_tiles[g[0]] = (e_t, None)
                    # pair on slots (1,2): one batched [2, qsh] activate
                    nc.scalar.activation(
                        e_t[:, :, :], st_b[:, :, :],
                        mybir.ActivationFunctionType.Exp, bias=nbias[:],
                    )
                else:
                    fx = None
                    for c, j in enumerate(g):
                        off = fast_off(j)
                        if j == 0 or j == ck - 1:
                            # pipeline-edge chunks: h-split to start the
                            # exp stream earlier (head) / drain it earlier
                            # (tail)
                            for h in range(nh):
                                sl = slice(h * FDIM, (h + 1) * FDIM)
                                nc.scalar.activation(
                                    e_t[:, c, sl], st_ap(j, sl),
                                    mybir.ActivationFunctionType.Exp, bias=nbias[:],
                                )
                        elif off is None:
                            nc.scalar.activation(
                                e_t[:, c, :], st_ap(j, slice(0, qsh)),
                                mybir.ActivationFunctionType.Exp, bias=nbias[:],
                            )
                        else:
                            comp = (
                                slice(FEW, qsh) if off == 0 else slice(0, qsh - FEW)
                            )
                            nc.scalar.activation(
                                e_t[:, c, comp], st_ap(j, comp),
                                mybir.ActivationFunctionType.Exp, bias=nbias[:],
                            )
                            # fast path: exp(s-1) ~= f16-bitcast(round(s*A+B))
                            fx = fxp.tile([P, FEW], dt.int16, name=f"fx{j}", tag="fx")
                            nc.vector.tensor_scalar(
                                fx[:], st_ap(j, slice(off, off + FEW)),
                                FE_A, FE_B,
                                mybir.AluOpType.mult, mybir.AluOpType.add,
                            )
                    e_tiles[g[0]] = (e_t, fx)

            def emit_mult_pv(g):
                # mask multiply: all-SBUF f16 -> DVE 2x mode. Per-chunk ops
                # because a compute group can straddle two em DMA pairs.
                e_t, fx = e_tiles.pop(g[0])
                pt = ptp.tile([P, 2, qsh], dt.float16, name=f"p{g[0]}", tag="p")
                for c, j in enumerate(g):
                    off = fast_off(j)
                    if j == 0 or j == ck - 1:
                        for h in range(nh):
                            sl = slice(h * FDIM, (h + 1) * FDIM)
                            nc.vector.tensor_mul(
                                pt[:, c, sl], e_t[:, c, sl],
                                mt_tiles[j // 2][:, j % 2, sl],
                            )
                    elif off is None:
                        nc.vector.tensor_mul(
                            pt[:, c, :], e_t[:, c, :], mt_tiles[j // 2][:, j % 2, :]
                        )
                    else:
                        comp = slice(FEW, qsh) if off == 0 else slice(0, qsh - FEW)
                        fsl = slice(off, off + FEW)
                        nc.vector.tensor_mul(
                            pt[:, c, comp], e_t[:, c, comp],
                            mt_tiles[j // 2][:, j % 2, comp],
                        )
                        nc.vector.tensor_mul(
                            pt[:, c, fsl], fx[:].bitcast(dt.float16),
                            mt_tiles[j // 2][:, j % 2, fsl],
                        )
                for c, j in enumerate(g):
                    vaj = va_sb[:, j * VF:j * VF + D + 1]
                    for h in range(nh):
                        sl = slice(h * FDIM, (h + 1) * FDIM)
                        nc.tensor.matmul(
                            ot_h[h][:, :], vaj, pt[:, c, sl],
                            start=(j == 0), stop=(j == ck - 1),
                            skip_group_check=True,
                        )

            qk_state = [0]

            def pump_qk(upto):
                while qk_state[0] < min(upto, ck):
                    j = qk_state[0]
                    if j % 2 == 0:
                        pp = j // 2
                        if pp > 0:
                            mt_tiles[pp] = mtp.tile(
                                [P, 2, qsh], dt.float16, name=f"mt{pp}", tag="mt"
                            )
                            nc.sync.dma_start(mt_tiles[pp][:], em_pairs[pp])
                        if 1 <= pp < nva:
                            nc.sync.dma_start(
                                va_sb[:, pp * vs:(pp + 1) * vs],
                                va[:, pp * vs:(pp + 1) * vs],
                            )
                    for si in kt_issue.pop(j, []):
                        a, b = kt_slices[si]
                        nc.gpsimd.memset(kt_sb[D:kp, a:b], 0.0)
                        nc.gpsimd.dma_start(kt_sb[0:D, a:b], kt[:, a:b])
                    emit_qk(j)
                    qk_state[0] += 1

            # emission: software-pipelined. Per triplet, each ACT is
            # immediately followed by the QK that recycles the ring slot it
            # just read (emission order defines the dependency tracker's
            # program semantics, so the slot-recycling QK must come *after*
            # its reader-ACT), and all of the next triplet's QKs precede
            # this triplet's PVs in the PE stream. PVs are gated on
            # mult <- ACT; if they sat ahead of the QKs in the in-order PE
            # queue they would head-block score production and starve
            # ScalarE (~1.3us/triplet measured).
            pump_qk(2)
            for gi in range(0, len(groups) - 1, 2):
                g_pair, g_single = groups[gi], groups[gi + 1]
                emit_act(g_pair)
                pump_qk(g_pair[-1] + 4)
                emit_act(g_single)
                pump_qk(g_single[0] + 4)
                emit_mult_pv(g_pair)
                emit_mult_pv(g_single)
            for g in groups[len(groups) - len(groups) % 2:]:
                emit_act(g)
                emit_mult_pv(g)

            # tail: ship numerator rows + denominator row; host divides.
            # Halves copy concurrently on ScalarE and VectorE, DMAs on two
            # independent queues.
            o_sb = tailp.tile([D + 1, qsh], dt.float16)
            for h in range(nh):
                sl = slice(h * FDIM, (h + 1) * FDIM)
                if h % 2 == 0:
                    nc.scalar.copy(o_sb[:, sl], ot_h[h][:, :])
                    nc.sync.dma_start(out[:, sl], o_sb[:, sl])
                else:
                    nc.vector.tensor_copy(o_sb[:, sl], ot_h[h][:, :])
                    nc.gpsimd.dma_start(out[:, sl], o_sb[:, sl])

    _split_excess_waits(nc)
    return nc


def _install_light_tail():
    """Tile's kernel tail is drain + 2 full all-engine butterfly barriers +
    sem clears (~11 us measured). For single-execution NEFFs the second
    barrier only guards sem-recycling across executions; drop it. The range
    sem-clears stay (cheap, keeps re-execution mostly sane)."""
    import concourse.tile as tile_mod
    from concourse.vector_clock import ScopedClock

    def _drain_and_barrier(self, tick_clock, wait_clock):
        nc = self.nc
        drain_inst = nc.sync.drain()
        wait_clock.add_sem_waits(
            drain_inst.ins, ScopedClock({None: tick_clock.global_clock})
        )
        assert self.sems is not None
        popped = nc._tile_sem_poison_stack.pop()
        assert popped is self._sem_poison

    tile_mod.TileContext._drain_and_barrier = _drain_and_barrier


def _prep_core_inputs(K, V, Q, m, core, qsh, nk, kp=P):
    scale = 1.0 / np.sqrt(np.float32(D))
    qs = slice(core * qsh, (core + 1) * qsh)
    ck = nk // P

    em = np.exp(np.ascontiguousarray(m[qs, :].T)).astype(np.float16)

    qt = np.ascontiguousarray(
        (Q[qs].astype(np.float32) * scale).T
    ).astype(np.float16)

    kt = np.ascontiguousarray(K.T).astype(np.float16)

    va = np.zeros((P, ck * VF), np.float16)
    va3 = va.reshape(P, ck, VF)
    va3[:, :, :D] = V.astype(np.float16).reshape(ck, P, D).transpose(1, 0, 2)
    va3[:, :, D] = np.float16(1.0)

    return {"em": em, "qt": qt, "kt": kt, "va": va}


def _get_nc(qsh, nk):
    key = (qsh, nk)
    if key not in _nc_cache:
        _install_tile_patch()
        _nc_cache[key] = _build_nc(qsh, nk)
    return _nc_cache[key]


def _run(K, V, Q, m, trace=False, n_cores=N_CORES, tmpdir=None):
    from concourse.bass_utils import run_bass_kernel_spmd

    K = np.asarray(K, dtype=np.float32)
    V = np.asarray(V, dtype=np.float32)
    Q = np.asarray(Q, dtype=np.float32)
    m = np.asarray(m, dtype=np.float32)
    nq, nk = m.shape
    qsh = nq // n_cores

    _install_tile_patch()
    nc = _get_nc(qsh, nk)
    in_maps = [
        _prep_core_inputs(K, V, Q, m, c, qsh, nk) for c in range(n_cores)
    ]
    res = run_bass_kernel_spmd(
        nc, in_maps, list(range(n_cores)), trace=trace, tmpdir=tmpdir
    )
    shards = []
    for c in range(n_cores):
        ot = res.results[c]["ot_out"].astype(np.float32)  # [D+1, qsh]
        shards.append((ot[:D] / ot[D:D + 1]).T)
    out = np.concatenate(shards, axis=0).astype(np.float32)
    return out, res


def kernel(**inputs):
    out, _ = _run(inputs["K"], inputs["V"], inputs["Q"], inputs["m"])
    return out


# revision 46
# speedup vs baseline: 1.1992x; 1.1992x over previous
"""Sparse-attention kernel for Trainium2, 8-core SPMD (queries sharded).

Computes out = softmax(Q @ K^T / sqrt(D) + m) @ V for
Q,K,V: [8192, 64] f32, m: [8192, 8192] f32.

Strategy (per core c over query shard q_c = rows [c*1024, (c+1)*1024)):
  Everything is computed in transposed (S^T) layout so that the exp output
  lands directly in the [key, query] orientation the PV matmul needs.

  Key idea vs the additive-mask formulation: softmax(s + m) uses
  exp(s + m) = exp(s) * exp(m), and the softmax ratio is shift-invariant.
  The host ships em = exp(m)^T in f16 (same bytes as m in f16), the device
  exps the *pure* QK scores straight out of PSUM (no mask add in PSUM at
  all), and the mask is applied as an all-SBUF f16 multiply on the DVE
  (2x mode), which is far off the critical path. This removes the PE
  identity-matmul and the DVE f32 PSUM add, and leaves PSUM holding only
  QK scores so ScalarE activations can batch chunks.

  Chunks are processed in [pair, single] triplets: chunks 3k/3k+1 land
  in the PSUM score tile st_b [128,2,1024] (4 banks), chunk 3k+2 in
  st_a [128,1024] (2 banks), + two O^T h-half accumulators (1 bank
  each) = 8 banks exactly. Each pair is ONE batched [2,1024] ACTIVATE, so
  ScalarE runs 1966+1114 = 3080ns per 3 chunks (vs 3x1114); the measured
  period is ~3104ns/triplet, pinned by the invariant chain
  [pair-ACT end -> two slot-recycling QKs -> next pair-ACT]. Two separate score
  tiles matter: the dependency tracker treats PSUM reads as RMW, totally
  ordering all accesses to one tile by emission order; the split lets the
  single-chain (QK->ACT_s) overlap the pair-chain (QK,QK->ACT_p).
  Emission is explicitly software-pipelined (QKs one triplet ahead of
  their group, each ACT immediately followed by the QK that recycles its
  slot) because the in-order PE queue otherwise head-blocks on PVs.

  Host-side sharding prep (layout/dtype only, plus exp(m)):
    em   = exp(m[q_c, :]).T      [8192, 1024] f16
    qt   = Q[q_c].T / 8          [64, 1024]   f16   (pad rows memset on-chip)
    kt   = K.T                   [64, 8192]   f16   (pad rows memset on-chip)
    va   = [V | 1]               [128, CK*66] f16   (col 64 of each chunk = 1
                                                     -> row 64 of O^T = sum(P))
  Device, per k-chunk j (128 keys):
    S^T[j]  = kt_j.T @ qt                    (PE -> PSUM score tile)
    E^T     = exp(S^T - 1)                   (ScalarE, PSUM -> SBUF f16)
    P^T[j]  = E^T[j] * em_j                  (DVE f16 multiply, 2x mode)
    O^T    += va_j.T @ P^T[j]                (PSUM [65, 2x512], accumulated)
  Ramp: qt leads the sync DMA queue, kt goes in fine-grained slices on the
  gpsimd queue (first QK only needs 32KB), em pairs + va slices stream on
  sync; zero-pad rows are memset on gpsimd instead of DMAing hosted zeros
  (the early wire is the ramp bottleneck). A short throwaway-matmul burst
  warms the PE HAM clock gate and hands off into real QK work. Tail:
  chunk 63's exp/mult are h-split, the two O^T h-halves drain through
  ScalarE/DVE copies + sync DMAs in parallel; the host divides num/sum.
"""

import numpy as np

P = 128
D = 64
NQ = 8192
NK = 8192
N_CORES = 8
VF = 66  # vaug chunk stride (65 cols used, padded for alignment)
FDIM = 512  # matmul moving free dim (one PSUM bank of f32)
NSLOT = 3  # PSUM score-ring depth (chunks in flight)
FEW = 256  # columns per single-chunk routed to the DVE fast-exp path
FE_A = 1024.0 / 0.6931471805599453  # f16 Schraudolph scale (2^10/ln2)
FE_B = 15 * 1024 - 47.0 - FE_A      # bias - sigma - the exp(-1) shift

_nc_cache = {}
_patched = [False]


def _install_tile_patch():
    """No-op placeholder kept for API stability (see _split_excess_waits)."""
    _patched[0] = True


def _split_excess_waits(nc, max_waits=1):
    """Walrus in this toolchain rejects instructions carrying more than one
    inline sync-wait command. Move excess waits onto same-engine NOPs
    inserted immediately before the instruction (the engine executes them
    in order, so the barrier semantics are preserved)."""
    import concourse.mybir as mybir

    for fn in nc.m.functions:
        for blk in fn.blocks:
            idx = 0
            while idx < len(blk.instructions):
                inst = blk.instructions[idx]
                si = inst.sync_info
                waits = list(si.on_wait) if si is not None and si.on_wait else []
                if len(waits) <= max_waits:
                    idx += 1
                    continue
                updates = list(si.on_update) if si.on_update else []
                keep = waits[-max_waits:]
                rest = waits[:-max_waits]
                inst.sync_info = mybir.SyncInfo(on_wait=keep, on_update=updates)
                n_nops = 0
                for i in range(0, len(rest), max_waits):
                    nop = mybir.InstNoOp(
                        name=nc.get_next_instruction_name(), ins=[], outs=[]
                    )
                    nop.engine = inst.engine
                    nop.sync_info = mybir.SyncInfo(
                        on_wait=rest[i:i + max_waits], on_update=[]
                    )
                    nc.register_instruction(nop)
                    blk.instructions.insert(idx + n_nops, nop)
                    n_nops += 1
                idx += n_nops + 1


def _build_nc(qsh, nk, mt_bufs=20, e_bufs=8, light_tail=True, kp=P):
    import concourse.bass as bass
    import concourse.mybir as mybir
    import concourse.tile as tile

    dt = mybir.dt
    ck = nk // P          # number of 128-key chunks
    npair = ck // 2       # em DMAs move two chunks at a time
    nh = qsh // FDIM      # number of 512-query column blocks
    nks = 16              # kt/va DMA split count (spread over first pairs)
    assert qsh % FDIM == 0 and nk % (2 * P) == 0 and nk % nks == 0 and (ck * VF) % nks == 0

    nc = bass.Bass()
    em = nc.declare_dram_parameter("em", [nk, qsh], dt.float16, isOutput=False)
    qt = nc.declare_dram_parameter("qt", [D, qsh], dt.float16, isOutput=False)
    kt = nc.declare_dram_parameter("kt", [D, nk], dt.float16, isOutput=False)
    va = nc.declare_dram_parameter("va", [P, ck * VF], dt.float16, isOutput=False)
    out = nc.declare_dram_parameter("ot_out", [D + 1, qsh], dt.float16, isOutput=True)

    em_pairs = em.rearrange("(pp c p) q -> pp p c q", c=2, p=P)  # [npair, 128, 2, qsh]

    if light_tail:
        _install_light_tail()

    # activation groups in a repeating [pair, single] triplet pattern:
    # pairs always land on ring slots (0,1) (tile st_b) and singles on
    # slot 2 (tile st_a), so every pair batches into one [2, qsh]
    # ACTIVATE: 3080ns of ScalarE work per 3 chunks vs 3x1114.
    groups = []
    for k in range(ck // 3):
        groups += [[3 * k, 3 * k + 1], [3 * k + 2]]
    groups += [[c] for c in range(3 * (ck // 3), ck)]

    with tile.TileContext(nc) as tc:
        with (
            tc.tile_pool(name="const", bufs=1) as cpool,
            tc.tile_pool(name="mtp", bufs=mt_bufs) as mtp,
            tc.tile_pool(name="ep", bufs=e_bufs) as epool,
            tc.tile_pool(name="fxp", bufs=3) as fxp,
            tc.tile_pool(name="ptp", bufs=e_bufs) as ptp,
            tc.tile_pool(name="tail", bufs=1) as tailp,
            tc.tile_pool(name="stp", bufs=1, space="PSUM") as stp,
            tc.tile_pool(name="otp", bufs=1, space="PSUM") as otp,
        ):
            # --- earliest DMAs first: nothing on the device gates these.
            # qt leads the sync queue (it gates the very first QK); the em
            # stream follows. kt/qt are host-padded to 128 rows (K=64
            # row-group matmuls keep the PE in its low-power half-array
            # mode -- measured 2x slower -- and an on-chip pad memset costs
            # ~7us of DVE time).
            qt_sb = cpool.tile([kp, qsh], dt.float16)
            nc.sync.dma_start(qt_sb[0:D, 0:FDIM], qt[:, 0:FDIM])
            nc.sync.dma_start(qt_sb[0:D, FDIM:qsh], qt[:, FDIM:qsh])

            mt_tiles = {}
            mt_tiles[0] = mtp.tile([P, 2, qsh], dt.float16, name="mt0", tag="mt")
            nc.sync.dma_start(mt_tiles[0][:], em_pairs[0])

            # kt slice plan: two 2-chunk slices up front (the first QK
            # only needs 32KB, not 0.5MB), then 4-chunk slices. Issued on
            # gpsimd ahead of the consuming chunk; va rides the sync queue
            # interleaved with the em pairs.
            kt_sb = cpool.tile([kp, nk], dt.float16)
            kt_slices = [(0, 3 * P), (3 * P, 4 * P)] + [
                (4 * P * i, 4 * P * (i + 1)) for i in range(1, ck // 4)
            ]
            kt_issue = {}  # chunk j -> kt slice indices (slices 0-2 below)
            for si in range(3, len(kt_slices)):
                kt_issue.setdefault(max(2, 4 * (si - 1) - 8), []).append(si)
            # DMA issues first, pad memsets after: the memsets are not
            # needed until the QKs (~13us) but each one delays the next
            # DMA issue on the in-order gpsimd queue
            for si in (0, 1, 2):
                a, b = kt_slices[si]
                nc.gpsimd.dma_start(kt_sb[0:D, a:b], kt[:, a:b])
            nc.gpsimd.memset(qt_sb[D:kp, :], 0.0)
            for si in (0, 1, 2):
                a, b = kt_slices[si]
                nc.gpsimd.memset(kt_sb[D:kp, a:b], 0.0)

            va_sb = cpool.tile([P, ck * VF], dt.float16)
            nva = 16
            vs = (ck * VF) // nva
            nc.sync.dma_start(va_sb[:, 0:vs], va[:, 0:vs])

            # --- warm-up: exp spline tables + PE HAM, riding the DMA ramp ---
            warm = cpool.tile([1, 2], dt.float32)
            nc.vector.memset(warm[:], 0.0)
            nc.scalar.activation(
                warm[:], warm[:], mybir.ActivationFunctionType.Exp
            )

            # per-partition bias vector holding the -1 softmax shift
            # (overflow headroom for the f16 exp products)
            nbias = cpool.tile([P, 1], dt.float32)
            nc.vector.memset(nbias[:], -1.0)

            # Score ring as TWO tiles: slot 0 (singles) and slots 1-2
            # (pairs). The dependency tracker treats PSUM reads as RMW, so
            # all accesses to one tile are totally ordered by emission;
            # separate tiles let the single-chain (QK -> ACT_s) overlap the
            # pair-chain (QK,QK -> ACT_p) instead of serializing PE against
            # ScalarE.
            st_a = stp.tile([P, qsh], dt.float32, name="st_a")      # 2 banks
            st_b = stp.tile([P, 2, qsh], dt.float32, name="st_b")   # 4 banks
            ot_h = [
                otp.tile([D + 1, FDIM], dt.float32, name="ot_h0"),
                otp.tile([D + 1, FDIM], dt.float32, name="ot_h1"),
            ]  # 1 bank each; separate tiles so the h0 drain chain doesn't
            # wait on h1's final PV (PSUM accesses are totally ordered
            # per tile)

            wz = cpool.tile([P, P], dt.float16)
            nc.vector.memset(wz[:], 0.0)
            for _ in range(10):
                nc.tensor.matmul(
                    st_a[:, 0:P], wz[:], wz[:],
                    start=True, stop=True, skip_group_check=True,
                )

            def st_ap(j, sl):
                s = j % NSLOT
                return st_a[:, sl] if s == 2 else st_b[:, s, sl]

            def emit_qk(j):
                # N=512 per matmul: a matmul dst may not cross a PSUM bank
                # boundary (hard HW constraint, verified), so each chunk's
                # scores take two FDIM-wide matmuls
                ktj = kt_sb[:, j * P:(j + 1) * P]
                for h in range(nh):
                    sl = slice(h * FDIM, (h + 1) * FDIM)
                    nc.tensor.matmul(
                        st_ap(j, sl), ktj, qt_sb[:, sl],
                        start=True, stop=True, skip_group_check=True,
                    )

            e_tiles = {}

            def fast_off(j):
                # single non-edge chunks offload FEW query-columns to a DVE
                # int16-Schraudolph exp; the offset alternates per triplet
                # so no query row concentrates too much approximation error
                # (A Schraudolph int16 fast-exp path on the DVE was
                # measured here: it saves ScalarE time on paper but gains
                # ~0 wall-clock in the fast clock state while spending most
                # of the 2e-2 error budget (1.78e-2) -- disabled.)
                return None

            def emit_act(g):
                e_t = epool.tile([P, 2, qsh], dt.float16, name=f"e{g[0]}", tag="e")
                if len(g) == 2:
                    e_tiles[g[0]] = (e_t, None)
                    # pair on slots (1,2): one batched [2, qsh] activate
                    nc.scalar.activation(
                        e_t[:, :, :], st_b[:, :, :],
                        mybir.ActivationFunctionType.Exp, bias=nbias[:],
                    )
                else:
                    fx = None
                    for c, j in enumerate(g):
                        off = fast_off(j)
                        if j == 0 or j == ck - 1:
                            # pipeline-edge chunks: h-split to start the
                            # exp stream earlier (head) / drain it earlier
                            # (tail)
                            for h in range(nh):
                                sl = slice(h * FDIM, (h + 1) * FDIM)
                                nc.scalar.activation(
                                    e_t[:, c, sl], st_ap(j, sl),
                                    mybir.ActivationFunctionType.Exp, bias=nbias[:],
                                )
                        elif off is None:
                            nc.scalar.activation(
                                e_t[:, c, :], st_ap(j, slice(0, qsh)),
                                mybir.ActivationFunctionType.Exp, bias=nbias[:],
                            )
                        else:
                            comp = (
                                slice(FEW, qsh) if off == 0 else slice(0, qsh - FEW)
                            )
                            nc.scalar.activation(
                                e_t[:, c, comp], st_ap(j, comp),
                                mybir.ActivationFunctionType.Exp, bias=nbias[:],
                            )
                            # fast path: exp(s-1) ~= f16-bitcast(round(s*A+B))
                            fx = fxp.tile([P, FEW], dt.int16, name=f"fx{j}", tag="fx")
                            nc.vector.tensor_scalar(
                                fx[:], st_ap(j, slice(off, off + FEW)),
                                FE_A, FE_B,
                                mybir.AluOpType.mult, mybir.AluOpType.add,
                            )
                    e_tiles[g[0]] = (e_t, fx)

            def emit_mult_pv(g):
                # mask multiply: all-SBUF f16 -> DVE 2x mode. Per-chunk ops
                # because a compute group can straddle two em DMA pairs.
                e_t, fx = e_tiles.pop(g[0])
                pt = ptp.tile([P, 2, qsh], dt.float16, name=f"p{g[0]}", tag="p")
                for c, j in enumerate(g):
                    off = fast_off(j)
                    if j == 0 or j == ck - 1:
                        for h in range(nh):
                            sl = slice(h * FDIM, (h + 1) * FDIM)
                            nc.vector.tensor_mul(
                                pt[:, c, sl], e_t[:, c, sl],
                                mt_tiles[j // 2][:, j % 2, sl],
                            )
                    elif off is None:
                        nc.vector.tensor_mul(
                            pt[:, c, :], e_t[:, c, :], mt_tiles[j // 2][:, j % 2, :]
                        )
                    else:
                        comp = slice(FEW, qsh) if off == 0 else slice(0, qsh - FEW)
                        fsl = slice(off, off + FEW)
                        nc.vector.tensor_mul(
                            pt[:, c, comp], e_t[:, c, comp],
                            mt_tiles[j // 2][:, j % 2, comp],
                        )
                        nc.vector.tensor_mul(
                            pt[:, c, fsl], fx[:].bitcast(dt.float16),
                            mt_tiles[j // 2][:, j % 2, fsl],
                        )
                for c, j in enumerate(g):
                    vaj = va_sb[:, j * VF:j * VF + D + 1]
                    for h in range(nh):
                        sl = slice(h * FDIM, (h + 1) * FDIM)
                        nc.tensor.matmul(
                            ot_h[h][:, :], vaj, pt[:, c, sl],
                            start=(j == 0), stop=(j == ck - 1),
                            skip_group_check=True,
                        )

            qk_state = [0]

            def pump_qk(upto):
                while qk_state[0] < min(upto, ck):
                    j = qk_state[0]
                    if j % 2 == 0:
                        pp = j // 2
                        if pp > 0:
                            mt_tiles[pp] = mtp.tile(
                                [P, 2, qsh], dt.float16, name=f"mt{pp}", tag="mt"
                            )
                            nc.sync.dma_start(mt_tiles[pp][:], em_pairs[pp])
                        if 1 <= pp < nva:
                            nc.sync.dma_start(
                                va_sb[:, pp * vs:(pp + 1) * vs],
                                va[:, pp * vs:(pp + 1) * vs],
                            )
                    for si in kt_issue.pop(j, []):
                        a, b = kt_slices[si]
                        nc.gpsimd.memset(kt_sb[D:kp, a:b], 0.0)
                        nc.gpsimd.dma_start(kt_sb[0:D, a:b], kt[:, a:b])
                    emit_qk(j)
                    qk_state[0] += 1

            # emission: software-pipelined. Per triplet, each ACT is
            # immediately followed by the QK that recycles the ring slot it
            # just read (emission order defines the dependency tracker's
            # program semantics, so the slot-recycling QK must come *after*
            # its reader-ACT), and all of the next triplet's QKs precede
            # this triplet's PVs in the PE stream. PVs are gated on
            # mult <- ACT; if they sat ahead of the QKs in the in-order PE
            # queue they would head-block score production and starve
            # ScalarE (~1.3us/triplet measured).
            pump_qk(2)
            for gi in range(0, len(groups) - 1, 2):
                g_pair, g_single = groups[gi], groups[gi + 1]
                emit_act(g_pair)
                pump_qk(g_pair[-1] + 4)
                emit_act(g_single)
                pump_qk(g_single[0] + 4)
                emit_mult_pv(g_pair)
                emit_mult_pv(g_single)
            for g in groups[len(groups) - len(groups) % 2:]:
                emit_act(g)
                emit_mult_pv(g)

            # tail: ship numerator rows + denominator row; host divides.
            # Halves copy concurrently on ScalarE and VectorE, DMAs on two
            # independent queues.
            o_sb = tailp.tile([D + 1, qsh], dt.float16)
            for h in range(nh):
                sl = slice(h * FDIM, (h + 1) * FDIM)
                if h % 2 == 0:
                    nc.scalar.copy(o_sb[:, sl], ot_h[h][:, :])
                    nc.sync.dma_start(out[:, sl], o_sb[:, sl])
                else:
                    nc.vector.tensor_copy(o_sb[:, sl], ot_h[h][:, :])
                    nc.gpsimd.dma_start(out[:, sl], o_sb[:, sl])

    _split_excess_waits(nc)
    return nc


def _install_light_tail():
    """Tile's kernel tail is drain + 2 full all-engine butterfly barriers +
    sem clears (~11 us measured). For single-execution NEFFs the second
    barrier only guards sem-recycling across executions; drop it. The range
    sem-clears stay (cheap, keeps re-execution mostly sane)."""
    import concourse.tile as tile_mod
    from concourse.vector_clock import ScopedClock

    def _drain_and_barrier(self, tick_clock, wait_clock):
        nc = self.nc
        drain_inst = nc.sync.drain()
        wait_clock.add_sem_waits(
            drain_inst.ins, ScopedClock({None: tick_clock.global_clock})
        )
        assert self.sems is not None
        popped = nc._tile_sem_poison_stack.pop()
        assert popped is self._sem_poison

    tile_mod.TileContext._drain_and_barrier = _drain_and_barrier


def _prep_core_inputs(K, V, Q, m, core, qsh, nk, kp=P):
    scale = 1.0 / np.sqrt(np.float32(D))
    qs = slice(core * qsh, (core + 1) * qsh)
    ck = nk // P

    em = np.exp(np.ascontiguousarray(m[qs, :].T)).astype(np.float16)

    qt = np.ascontiguousarray(
        (Q[qs].astype(np.float32) * scale).T
    ).astype(np.float16)

    kt = np.ascontiguousarray(K.T).astype(np.float16)

    va = np.zeros((P, ck * VF), np.float16)
    va3 = va.reshape(P, ck, VF)
    va3[:, :, :D] = V.astype(np.float16).reshape(ck, P, D).transpose(1, 0, 2)
    va3[:, :, D] = np.float16(1.0)

    return {"em": em, "qt": qt, "kt": kt, "va": va}


def _get_nc(qsh, nk):
    key = (qsh, nk)
    if key not in _nc_cache:
        _install_tile_patch()
        _nc_cache[key] = _build_nc(qsh, nk)
    return _nc_cache[key]


def _run(K, V, Q, m, trace=False, n_cores=N_CORES, tmpdir=None):
    from concourse.bass_utils import run_bass_kernel_spmd

    K = np.asarray(K, dtype=np.float32)
    V = np.asarray(V, dtype=np.float32)
    Q = np.asarray(Q, dtype=np.float32)
    m = np.asarray(m, dtype=np.float32)
    nq, nk = m.shape
    qsh = nq // n_cores

    _install_tile_patch()
    nc = _get_nc(qsh, nk)
    in_maps = [
        _prep_core_inputs(K, V, Q, m, c, qsh, nk) for c in range(n_cores)
    ]
    res = run_bass_kernel_spmd(
        nc, in_maps, list(range(n_cores)), trace=trace, tmpdir=tmpdir
    )
    shards = []
    for c in range(n_cores):
        ot = res.results[c]["ot_out"].astype(np.float32)  # [D+1, qsh]
        shards.append((ot[:D] / ot[D:D + 1]).T)
    out = np.concatenate(shards, axis=0).astype(np.float32)
    return out, res


def kernel(**inputs):
    out, _ = _run(inputs["K"], inputs["V"], inputs["Q"], inputs["m"])
    return out
